# revision 10
# baseline (speedup 1.0000x reference)
"""Multi-head causal attention (B=4, T=2048, D=1024, H=16, HS=64) on 8 TRN2 cores.

Sharding: tensor-parallel over heads (2 heads/core) for QKV+attention, then
AllToAlls redistribute per-head context to token-parallel layout for the output
projection. Two AllToAlls per batch (8 total, 128-token sub-blocks), so the
projection of one half-batch overlaps the attention of the next.

Engine assignment discipline (v1 was ACT-bound, v2 was LDWEIGHTS-bound):
  - ACT (scalar) runs ONLY the softmax exp -> single activation-table load.
  - All PSUM->SBUF copies and the context normalization run on DVE.
  - Softmax reciprocal: DVE reciprocal_approx_fast on the PSUM denominator row
    (partition 64), rounded to f32r, partition-broadcast by a K=1 PE matmul
    whose stationary ones-row also lives on partition 64 (tile_position 64,0).
  - V is computed transpose-free (x^T t-tile stationary, Wv moving -> [token,
    head*hs]); its 32 small weight-loads per chunk hide under the q/k N=512
    streams by interleaving the matmuls.
  - Output projection keeps the received context STATIONARY (8 weight-loads
    per group) and streams Wp; output is token-major so no transpose.
  - Normalization of chunk c is emitted inside chunk c+1's first j-tile so
    the PE/ACT queues never drain at chunk boundaries.

All matmuls bf16 with fp32 PSUM accumulation; softmax without max-subtraction
(|scores| <= ~8 for these inputs, exp is safe in fp32).
"""
import numpy as np

import concourse.bass as bass
import concourse.tile as tile
from concourse import bacc, mybir
from concourse.bass_utils import run_bass_kernel_spmd

f32 = mybir.dt.float32
f32r = mybir.dt.float32r
bf16 = mybir.dt.bfloat16

B, D, H, HS = 4, 1024, 16, 64
N_CORES = 8
HPC = H // N_CORES          # heads per core
QC = 512                    # q-chunk width
KT = 128                    # k-tile width
ND = D // 128               # din tiles

DT_NAME = "bf16"            # "f32r" | "bf16" | "f32"


def _np_dt(dt):
    import ml_dtypes
    return {f32: np.float32, f32r: np.float32, bf16: ml_dtypes.bfloat16}[dt]


def build_nc(T=2048, dt_name=DT_NAME):
    DT = {"f32r": f32r, "bf16": bf16, "f32": f32}[dt_name]
    BT = B * T
    SL = BT // N_CORES              # tokens per core in phase C (1024)
    NQC = T // QC                   # q-chunks per batch (4)
    NTB = T // KT                   # k-tiles per batch (16)
    NG = 2 * B                      # two AllToAlls per batch
    TB = 128                        # token sub-block (a2a slot width)
    assert NQC == 4 and SL == NG * TB

    nc = bacc.Bacc("TRN2", target_bir_lowering=False, debug=False,
                   num_devices=N_CORES)

    xt_d = nc.dram_tensor("xt", [D, BT], DT, kind="ExternalInput").ap()
    wq_d = nc.dram_tensor("wq", [D, 128], DT, kind="ExternalInput").ap()
    wk_d = nc.dram_tensor("wk", [D, 128], DT, kind="ExternalInput").ap()
    wv_d = nc.dram_tensor("wv", [D, 128], DT, kind="ExternalInput").ap()
    wp_d = nc.dram_tensor("wp", [D, D], DT, kind="ExternalInput").ap()
    bp_d = nc.dram_tensor("bp", [128, D], f32, kind="ExternalInput").ap()
    tril_d = nc.dram_tensor("triu", [128, 128], DT, kind="ExternalInput").ap()
    ones1_d = nc.dram_tensor("ones1", [65, 64], f32r, kind="ExternalInput").ap()
    onesm_d = nc.dram_tensor("onesm", [128, NTB], DT, kind="ExternalInput").ap()
    out_d = nc.dram_tensor("outT", [SL, D], f32, kind="ExternalOutput").ap()

    EXP = mybir.ActivationFunctionType.Exp

    with tile.TileContext(nc) as tc:
        with (
            tc.tile_pool(name="wts", bufs=1) as wts,
            tc.tile_pool(name="acts", bufs=1) as acts,
            tc.tile_pool(name="dram", bufs=1, space="DRAM") as dram,
        ):
            # ---- persistent loads ----
            wq_sb, wk_sb, wv_sb = [], [], []
            for j in range(ND):
                for lst, dd, nm in ((wq_sb, wq_d, "wq"), (wk_sb, wk_d, "wk"),
                                    (wv_sb, wv_d, "wv")):
                    t = wts.tile([128, 128], DT, name=f"{nm}{j}", tag=f"{nm}{j}")
                    nc.sync.dma_start(t[:], dd[j * 128:(j + 1) * 128, :])
                    lst.append(t)
            triu_sb = wts.tile([128, 128], DT, name="triu", tag="triu")
            nc.sync.dma_start(triu_sb[:], tril_d[:])
            ones1_sb = wts.tile([65, 64], f32r, name="ones1", tag="ones1")
            nc.sync.dma_start(ones1_sb[:], ones1_d[:])
            onesm_sb = wts.tile([128, NTB], DT, name="onesm", tag="onesm")
            nc.sync.dma_start(onesm_sb[:], onesm_d[:])

            a2a_in = [dram.tile([N_CORES, 128, TB], DT, name=f"a2ai{g}")
                      for g in range(NG)]
            a2a_out = [dram.tile([N_CORES, 128, TB], DT, name=f"a2ao{g}")
                       for g in range(NG)]

            # per-batch activation tensors
            qT, kT, vA = [], [], []
            for b in range(B):
                qT.append(acts.tile([128, T], DT, name=f"qT{b}", tag=f"qT{b}"))
                kT.append(acts.tile([128, T], DT, name=f"kT{b}", tag=f"kT{b}"))
                vA.append(acts.tile([128, NTB * 130], DT, name=f"vA{b}",
                                    tag=f"vA{b}"))

            # output-projection weights load late so phase A's x-stream gets
            # the DMA queues first
            wp_sb = []
            for j in range(ND):
                t = wts.tile([128, D], DT, name=f"wp{j}", tag=f"wp{j}")
                nc.sync.dma_start(t[:], wp_d[j * 128:(j + 1) * 128, :])
                wp_sb.append(t)
            bp_sb = wts.tile([128, D], f32, name="bpr", tag="bpr")
            nc.sync.dma_start(bp_sb[:], bp_d[:])

            with (
                tc.tile_pool(name="pA", bufs=2) as pA,
                tc.tile_pool(name="pB", bufs=3) as pB,
            ):
                psA_ctx = tc.tile_pool(name="psA", bufs=2, space="PSUM")
                psA = psA_ctx.__enter__()
                psB = psAV = None          # opened after psA closes
                _ps_ctxs = []

                def phase_a(b):
                    # ones columns of the augmented-V slots ([v0|1|v1|1])
                    v3 = vA[b][:].rearrange("p (t c) -> p t c", c=130)
                    nc.vector.tensor_copy(v3[:, :, 64], onesm_sb[:])
                    nc.vector.tensor_copy(v3[:, :, 129], onesm_sb[:])
                    for ch in range(NQC):
                        i0 = b * T + ch * QC
                        xt = []
                        for j in range(ND):
                            t = pA.tile([128, QC], DT, name=f"x{j}",
                                        tag=f"x{j}", bufs=2)
                            nc.sync.dma_start(
                                t[:], xt_d[j * 128:(j + 1) * 128, i0:i0 + QC])
                            xt.append(t)
                        ppq = psA.tile([128, QC], f32, name="ppq", tag="ppq",
                                       bufs=2)
                        ppk = psA.tile([128, QC], f32, name="ppk", tag="ppk",
                                       bufs=2)
                        vp = [psA.tile([128, 128], f32, name=f"vp{tt}",
                                       tag=f"vp{tt}", bufs=1)
                              for tt in range(4)]
                        # interleave: v's small ldweights hide under the q/k
                        # N=512 streams
                        for j in range(ND):
                            st, sp = (j == 0), (j == ND - 1)
                            nc.tensor.matmul(ppq[:], wq_sb[j][:], xt[j][:],
                                             start=st, stop=sp)
                            nc.tensor.matmul(
                                vp[0][:], xt[j][:, 0:128], wv_sb[j][:],
                                start=st, stop=sp)
                            nc.tensor.matmul(ppk[:], wk_sb[j][:], xt[j][:],
                                             start=st, stop=sp)
                            for tt in range(1, 4):
                                nc.tensor.matmul(
                                    vp[tt][:],
                                    xt[j][:, tt * 128:(tt + 1) * 128],
                                    wv_sb[j][:], start=st, stop=sp)
                        sl = slice(ch * QC, (ch + 1) * QC)
                        nc.vector.tensor_copy(qT[b][:, sl], ppq[:])
                        nc.vector.tensor_copy(kT[b][:, sl], ppk[:])
                        for tt in range(4):
                            slot = (ch * 4 + tt) * 130
                            dst3 = vA[b][:, slot:slot + 130].rearrange(
                                "p (s c) -> p s c", c=65)[:, :, 0:64]
                            src3 = vp[tt][:].rearrange("p (s c) -> p s c",
                                                       c=64)
                            nc.vector.tensor_copy(dst3, src3)

                pending = [None]

                def flush_norm():
                    if pending[0] is None:
                        return
                    pb_, pqc, pav = pending[0]
                    pending[0] = None
                    for h in range(HPC):
                        # hop the PSUM denominator row (partition 64) to
                        # partition 0 with a plain DVE copy (HW-proven);
                        # reciprocal_approx_fast requires base partition 0
                        den = pB.tile([1, QC], f32, name=f"den{h}",
                                      tag=f"den{h}", bufs=2)
                        nc.vector.tensor_copy(den[:], pav[h][64:65, :])
                        rec = pB.tile([1, QC], f32, name=f"rec{h}",
                                      tag=f"rec{h}", bufs=2)
                        nc.vector.reciprocal_approx_fast(rec[:], den[:])
                        recr = pB.tile([1, QC], f32r, name=f"recr{h}",
                                       tag=f"recr{h}", bufs=2)
                        nc.vector.tensor_copy(recr[:], rec[:])
                        bcp = psB.tile([64, QC], f32, name=f"bcp{h}",
                                       tag="scb", bufs=2)
                        nc.tensor.matmul(bcp[:], ones1_sb[0:1, :],
                                         recr[:],
                                         start=True, stop=True)
                        bcs = pB.tile([64, QC], f32, name=f"bcs{h}",
                                      tag=f"bcs{h}", bufs=2)
                        nc.vector.tensor_copy(bcs[:], bcp[:])
                        ctx = pB.tile([64, QC], DT, name=f"ctx{h}",
                                      tag=f"ctx{h}", bufs=2)
                        nc.vector.tensor_mul(ctx[:], pav[h][0:64, :], bcs[:])
                        # 4 token sub-blocks to their a2a slots
                        for lt in range(4):
                            l = 4 * pqc + lt
                            g = 2 * pb_ + l // 8
                            nc.sync.dma_start(
                                a2a_in[g][l % 8, h * 64:(h + 1) * 64, :],
                                ctx[:, lt * TB:(lt + 1) * TB])
                    if pqc % 2 == 1:
                        g = 2 * pb_ + pqc // 2
                        nc.gpsimd.collective_compute(
                            "AllToAll", mybir.AluOpType.bypass,
                            replica_groups=[list(range(N_CORES))],
                            ins=[a2a_in[g].opt()], outs=[a2a_out[g].opt()])

                def attn_chunk(b, qc):
                    av = [psAV.tile([65, QC], f32, name=f"av{h}",
                                    tag=f"av{h}", bufs=2)
                          for h in range(HPC)]
                    nj = 4 * qc + 4
                    for j in range(nj):
                        jr = j - 4 * qc
                        off = max(jr, 0) * 128
                        w = QC - off
                        qsl = slice(qc * QC + off, (qc + 1) * QC)
                        # both heads' scores into one 2-bank psum tile
                        sc = psB.tile([128, 2 * QC], f32, name="scb",
                                      tag="scb", bufs=2)
                        for h in range(HPC):
                            hp = slice(h * 64, (h + 1) * 64)
                            nc.tensor.matmul(
                                sc[:, h * QC:h * QC + w],
                                kT[b][hp, j * 128:(j + 1) * 128],
                                qT[b][hp, qsl], start=True, stop=True)
                        # one exp for both heads via a strided AP
                        e = pB.tile([128, 2 * w], DT, name="exb",
                                    tag="exb", bufs=4)
                        sc3 = sc[:].rearrange("p (two q) -> p two q",
                                              two=2)[:, :, 0:w]
                        e3 = e[:].rearrange("p (two q) -> p two q", two=2)
                        nc.scalar.activation(e3, sc3, EXP,
                                             scale=1.0 / np.sqrt(HS))
                        if jr >= 0:
                            for h in range(HPC):
                                nc.vector.tensor_mul(
                                    e[:, h * w:h * w + 128],
                                    e[:, h * w:h * w + 128], triu_sb[:])
                        if j == 0:
                            flush_norm()
                        for h in range(HPC):
                            lhs = vA[b][:, j * 130 + h * 65:
                                        j * 130 + h * 65 + 65]
                            nc.tensor.matmul(av[h][:, off:QC], lhs,
                                             e[:, h * w:(h + 1) * w],
                                             start=(j == 0),
                                             stop=(j == nj - 1))
                    pending[0] = (b, qc, av)

                def proj_group(g, pC):
                    # context tiles stationary (8 ldweights), Wp moving
                    cx = []
                    for j in range(ND):
                        t = pC.tile([128, TB], DT, name=f"cx{j}",
                                    tag=f"cx{j}", bufs=2)
                        nc.sync.dma_start(t[:], a2a_out[g][j])
                        cx.append(t)
                    ops = []
                    for half in range(2):
                        op = psB.tile([128, 512], f32, name=f"op{half}",
                                      tag="scb", bufs=2)
                        for j in range(ND):
                            nc.tensor.matmul(
                                op[:], cx[j][:],
                                wp_sb[j][:, half * 512:(half + 1) * 512],
                                start=(j == 0), stop=(j == ND - 1))
                        ops.append(op)
                    os_ = pC.tile([128, D], f32, name="os", tag="os", bufs=2)
                    for half in range(2):
                        nc.vector.tensor_add(
                            os_[:, half * 512:(half + 1) * 512], ops[half][:],
                            bp_sb[:, half * 512:(half + 1) * 512])
                    nc.sync.dma_start(out_d[g * TB:(g + 1) * TB, :], os_[:])

                with tc.tile_pool(name="pC", bufs=2) as pC:
                    with nc.named_scope("phA"):
                        for b in range(B):
                            phase_a(b)
                    psA_ctx.__exit__(None, None, None)
                    for nm in ("psB", "psAV"):
                        c = tc.tile_pool(name=nm, bufs=2, space="PSUM")
                        _ps_ctxs.append(c)
                    psB = _ps_ctxs[0].__enter__()
                    psAV = _ps_ctxs[1].__enter__()
                    for b in range(B):
                        with nc.named_scope(f"phB{b}"):
                            for qc in range(NQC):
                                attn_chunk(b, qc)
                                if qc == 1 and b >= 1:
                                    with nc.named_scope(f"phC{2*b-1}"):
                                        proj_group(2 * b - 1, pC)
                                if qc == 3:
                                    with nc.named_scope(f"phC{2*b}"):
                                        proj_group(2 * b, pC)
                    flush_norm()
                    with nc.named_scope(f"phC{NG-1}"):
                        proj_group(NG - 1, pC)
                    for c in reversed(_ps_ctxs):
                        c.__exit__(None, None, None)

    nc.compile()
    return nc


def prep_inputs(x, Wq, Wk, Wv, Wp, bp, T, dt_name=DT_NAME):
    """Host-side sharding/layout prep. Returns in_maps for the 8 cores."""
    DT = {"f32r": f32r, "bf16": bf16, "f32": f32}[dt_name]
    ndt = _np_dt(DT)
    BT = B * T
    NTB = T // KT

    x = np.asarray(x, np.float32)
    Wq = np.asarray(Wq, np.float32)
    Wk = np.asarray(Wk, np.float32)
    Wv = np.asarray(Wv, np.float32)
    Wp = np.asarray(Wp, np.float32)
    bp = np.asarray(bp, np.float32)

    xt = np.ascontiguousarray(x.reshape(BT, D).T).astype(ndt)
    wp = np.ascontiguousarray(Wp.T).astype(ndt)
    bpr = np.ascontiguousarray(np.broadcast_to(bp.reshape(1, D), (128, D)))
    triu = np.triu(np.ones((128, 128), np.float32)).astype(ndt)
    ones1 = np.ones((65, 64), np.float32)
    onesm = np.ones((128, NTB), np.float32).astype(ndt)

    def wslice(W, c):
        # [H, D, HS] heads 2c,2c+1 -> [D, 128] as [d, (h_local, e)]
        return np.ascontiguousarray(
            W[2 * c:2 * c + 2].transpose(1, 0, 2).reshape(D, 2 * HS)).astype(ndt)

    in_maps = []
    for c in range(N_CORES):
        in_maps.append({
            "xt": xt, "wq": wslice(Wq, c), "wk": wslice(Wk, c),
            "wv": wslice(Wv, c), "wp": wp, "bp": bpr,
            "triu": triu, "ones1": ones1, "onesm": onesm,
        })
    return in_maps


def gather_out(results, T):
    """results[c]["outT"] is [SL, D] token-major; core c's row block g holds
    token sub-block (b= g//2, l = (g%2)*8 + c) of 128 tokens."""
    BT = B * T
    full = np.empty((BT, D), np.float32)
    for c in range(N_CORES):
        o = np.asarray(results[c]["outT"])
        for g in range(2 * B):
            b, hh = g // 2, g % 2
            t0 = b * T + (hh * 8 + c) * 128
            full[t0:t0 + 128, :] = o[g * 128:(g + 1) * 128, :]
    return full.reshape(B, T, D)


_NC_CACHE = {}


def kernel(x, Wq, Wk, Wv, Wp, bp):
    T = np.asarray(x).shape[1]
    key = (T, DT_NAME)
    if key not in _NC_CACHE:
        _NC_CACHE[key] = build_nc(T, DT_NAME)
    nc = _NC_CACHE[key]
    in_maps = prep_inputs(x, Wq, Wk, Wv, Wp, bp, T, DT_NAME)
    res = run_bass_kernel_spmd(nc, in_maps, list(range(N_CORES)))
    return np.ascontiguousarray(
        gather_out(res.results, T).astype(np.float32))


# revision 13
# speedup vs baseline: 1.0134x; 1.0134x over previous
"""Multi-head causal attention (B=4, T=2048, D=1024, H=16, HS=64) on 8 TRN2 cores.

Sharding: tensor-parallel over heads (2 heads/core) for QKV+attention, then
AllToAlls redistribute per-head context to token-parallel layout for the output
projection. Two AllToAlls per batch (8 total, 128-token sub-blocks), so the
projection of one half-batch overlaps the attention of the next.

Engine assignment discipline (v1 was ACT-bound, v2 was LDWEIGHTS-bound):
  - ACT (scalar) runs ONLY the softmax exp -> single activation-table load.
  - All PSUM->SBUF copies and the context normalization run on DVE.
  - Softmax reciprocal: DVE reciprocal_approx_fast on the PSUM denominator row
    (partition 64), rounded to f32r, partition-broadcast by a K=1 PE matmul
    whose stationary ones-row also lives on partition 64 (tile_position 64,0).
  - V is computed transpose-free (x^T t-tile stationary, Wv moving -> [token,
    head*hs]); its 32 small weight-loads per chunk hide under the q/k N=512
    streams by interleaving the matmuls.
  - Output projection keeps the received context STATIONARY (8 weight-loads
    per group) and streams Wp; output is token-major so no transpose.
  - Normalization of chunk c is emitted inside chunk c+1's first j-tile so
    the PE/ACT queues never drain at chunk boundaries.

All matmuls bf16 with fp32 PSUM accumulation; softmax without max-subtraction
(|scores| <= ~8 for these inputs, exp is safe in fp32).
"""
import numpy as np

import concourse.bass as bass
import concourse.tile as tile
from concourse import bacc, mybir
from concourse.bass_utils import run_bass_kernel_spmd

f32 = mybir.dt.float32
f32r = mybir.dt.float32r
bf16 = mybir.dt.bfloat16

B, D, H, HS = 4, 1024, 16, 64
N_CORES = 8
HPC = H // N_CORES          # heads per core
QC = 512                    # q-chunk width
KT = 128                    # k-tile width
ND = D // 128               # din tiles

DT_NAME = "bf16"            # "f32r" | "bf16" | "f32"


def _np_dt(dt):
    import ml_dtypes
    return {f32: np.float32, f32r: np.float32, bf16: ml_dtypes.bfloat16}[dt]


def build_nc(T=2048, dt_name=DT_NAME):
    DT = {"f32r": f32r, "bf16": bf16, "f32": f32}[dt_name]
    BT = B * T
    SL = BT // N_CORES              # tokens per core in phase C (1024)
    NQC = T // QC                   # q-chunks per batch (4)
    NTB = T // KT                   # k-tiles per batch (16)
    NG = 2 * B                      # two AllToAlls per batch
    TB = 128                        # token sub-block (a2a slot width)
    assert NQC == 4 and SL == NG * TB

    nc = bacc.Bacc("TRN2", target_bir_lowering=False, debug=False,
                   num_devices=N_CORES)

    xt_d = nc.dram_tensor("xt", [D, BT], DT, kind="ExternalInput").ap()
    wq_d = nc.dram_tensor("wq", [D, 128], DT, kind="ExternalInput").ap()
    wk_d = nc.dram_tensor("wk", [D, 128], DT, kind="ExternalInput").ap()
    wv_d = nc.dram_tensor("wv", [D, 128], DT, kind="ExternalInput").ap()
    wp_d = nc.dram_tensor("wp", [D, D], DT, kind="ExternalInput").ap()
    bp_d = nc.dram_tensor("bp", [128, D], f32, kind="ExternalInput").ap()
    tril_d = nc.dram_tensor("triu", [128, 128], DT, kind="ExternalInput").ap()
    ones1_d = nc.dram_tensor("ones1", [65, 64], f32r, kind="ExternalInput").ap()
    onesm_d = nc.dram_tensor("onesm", [128, NTB], DT, kind="ExternalInput").ap()
    out_d = nc.dram_tensor("outT", [SL, D], f32, kind="ExternalOutput").ap()

    EXP = mybir.ActivationFunctionType.Exp

    with tile.TileContext(nc) as tc:
        with (
            tc.tile_pool(name="wts", bufs=1) as wts,
            tc.tile_pool(name="acts", bufs=1) as acts,
            tc.tile_pool(name="dram", bufs=1, space="DRAM") as dram,
        ):
            # ---- persistent loads ----
            wq_sb, wk_sb, wv_sb = [], [], []
            for j in range(ND):
                for lst, dd, nm in ((wq_sb, wq_d, "wq"), (wk_sb, wk_d, "wk"),
                                    (wv_sb, wv_d, "wv")):
                    t = wts.tile([128, 128], DT, name=f"{nm}{j}", tag=f"{nm}{j}")
                    nc.sync.dma_start(t[:], dd[j * 128:(j + 1) * 128, :])
                    lst.append(t)
            triu_sb = wts.tile([128, 128], DT, name="triu", tag="triu")
            nc.sync.dma_start(triu_sb[:], tril_d[:])
            ones1_sb = wts.tile([65, 64], f32r, name="ones1", tag="ones1")
            nc.sync.dma_start(ones1_sb[:], ones1_d[:])
            onesm_sb = wts.tile([128, NTB], DT, name="onesm", tag="onesm")
            nc.sync.dma_start(onesm_sb[:], onesm_d[:])

            a2a_in = [dram.tile([N_CORES, 128, TB], DT, name=f"a2ai{g}")
                      for g in range(NG)]
            a2a_out = [dram.tile([N_CORES, 128, TB], DT, name=f"a2ao{g}")
                       for g in range(NG)]

            # per-batch activation tensors
            qT, kT, vA = [], [], []
            for b in range(B):
                qT.append(acts.tile([128, T], DT, name=f"qT{b}", tag=f"qT{b}"))
                kT.append(acts.tile([128, T], DT, name=f"kT{b}", tag=f"kT{b}"))
                vA.append(acts.tile([128, NTB * 130], DT, name=f"vA{b}",
                                    tag=f"vA{b}"))

            # output-projection weights load late so phase A's x-stream gets
            # the DMA queues first
            wp_sb = []
            for j in range(ND):
                t = wts.tile([128, D], DT, name=f"wp{j}", tag=f"wp{j}")
                nc.sync.dma_start(t[:], wp_d[j * 128:(j + 1) * 128, :])
                wp_sb.append(t)
            bp_sb = wts.tile([128, D], f32, name="bpr", tag="bpr")
            nc.sync.dma_start(bp_sb[:], bp_d[:])

            with (
                tc.tile_pool(name="pA", bufs=2) as pA,
                tc.tile_pool(name="pB", bufs=3) as pB,
            ):
                psA_ctx = tc.tile_pool(name="psA", bufs=2, space="PSUM")
                psA = psA_ctx.__enter__()
                psB = psAV = None          # opened after psA closes
                _ps_ctxs = []

                def phase_a(b):
                    # ones columns of the augmented-V slots ([v0|1|v1|1])
                    v3 = vA[b][:].rearrange("p (t c) -> p t c", c=130)
                    nc.vector.tensor_copy(v3[:, :, 64], onesm_sb[:])
                    nc.vector.tensor_copy(v3[:, :, 129], onesm_sb[:])
                    for ch in range(NQC):
                        i0 = b * T + ch * QC
                        # one 3D DMA for all 8 din-tiles of the chunk
                        xtl = pA.tile([128, ND * QC], DT, name="xtl",
                                      tag="xtl", bufs=2)
                        nc.sync.dma_start(
                            xtl[:].rearrange("p (j c) -> p j c", j=ND),
                            xt_d[:, i0:i0 + QC].rearrange(
                                "(j p) c -> p j c", p=128))
                        xt = [xtl[:, j * QC:(j + 1) * QC] for j in range(ND)]
                        ppq = psA.tile([128, QC], f32, name="ppq", tag="ppq",
                                       bufs=2)
                        ppk = psA.tile([128, QC], f32, name="ppk", tag="ppk",
                                       bufs=2)
                        vp = [psA.tile([128, 128], f32, name=f"vp{tt}",
                                       tag=f"vp{tt}", bufs=1)
                              for tt in range(4)]
                        # interleave: v's small ldweights hide under the q/k
                        # N=512 streams
                        for j in range(ND):
                            st, sp = (j == 0), (j == ND - 1)
                            nc.tensor.matmul(ppq[:], wq_sb[j][:], xt[j],
                                             start=st, stop=sp)
                            nc.tensor.matmul(
                                vp[0][:], xt[j][:, 0:128], wv_sb[j][:],
                                start=st, stop=sp)
                            nc.tensor.matmul(ppk[:], wk_sb[j][:], xt[j],
                                             start=st, stop=sp)
                            for tt in range(1, 4):
                                nc.tensor.matmul(
                                    vp[tt][:],
                                    xt[j][:, tt * 128:(tt + 1) * 128],
                                    wv_sb[j][:], start=st, stop=sp)
                        sl = slice(ch * QC, (ch + 1) * QC)
                        nc.vector.tensor_copy(qT[b][:, sl], ppq[:])
                        nc.vector.tensor_copy(kT[b][:, sl], ppk[:])
                        for tt in range(4):
                            slot = (ch * 4 + tt) * 130
                            dst3 = vA[b][:, slot:slot + 130].rearrange(
                                "p (s c) -> p s c", c=65)[:, :, 0:64]
                            src3 = vp[tt][:].rearrange("p (s c) -> p s c",
                                                       c=64)
                            nc.vector.tensor_copy(dst3, src3)

                pending = [None]

                def flush_norm():
                    if pending[0] is None:
                        return
                    pb_, pqc, pav = pending[0]
                    pending[0] = None
                    for h in range(HPC):
                        # hop the PSUM denominator row (partition 64) to
                        # partition 0 with a plain DVE copy (HW-proven);
                        # reciprocal_approx_fast requires base partition 0
                        den = pB.tile([1, QC], f32, name=f"den{h}",
                                      tag=f"den{h}", bufs=2)
                        nc.vector.tensor_copy(den[:], pav[h][64:65, :])
                        rec = pB.tile([1, QC], f32, name=f"rec{h}",
                                      tag=f"rec{h}", bufs=2)
                        nc.vector.reciprocal_approx_fast(rec[:], den[:])
                        recr = pB.tile([1, QC], f32r, name=f"recr{h}",
                                       tag=f"recr{h}", bufs=2)
                        nc.vector.tensor_copy(recr[:], rec[:])
                        bcp = psB.tile([64, QC], f32, name=f"bcp{h}",
                                       tag="scb", bufs=2)
                        nc.tensor.matmul(bcp[:], ones1_sb[0:1, :],
                                         recr[:],
                                         start=True, stop=True)
                        bcs = pB.tile([64, QC], f32, name=f"bcs{h}",
                                      tag=f"bcs{h}", bufs=2)
                        nc.vector.tensor_copy(bcs[:], bcp[:])
                        ctx = pB.tile([64, QC], DT, name=f"ctx{h}",
                                      tag=f"ctx{h}", bufs=2)
                        nc.vector.tensor_mul(ctx[:], pav[h][0:64, :], bcs[:])
                        # 4 token sub-blocks to their a2a slots
                        for lt in range(4):
                            l = 4 * pqc + lt
                            g = 2 * pb_ + l // 8
                            nc.sync.dma_start(
                                a2a_in[g][l % 8, h * 64:(h + 1) * 64, :],
                                ctx[:, lt * TB:(lt + 1) * TB])
                    if pqc % 2 == 1:
                        g = 2 * pb_ + pqc // 2
                        nc.gpsimd.collective_compute(
                            "AllToAll", mybir.AluOpType.bypass,
                            replica_groups=[list(range(N_CORES))],
                            ins=[a2a_in[g].opt()], outs=[a2a_out[g].opt()])

                def attn_chunk(b, qc):
                    av = [psAV.tile([65, QC], f32, name=f"av{h}",
                                    tag=f"av{h}", bufs=2)
                          for h in range(HPC)]
                    nj = 4 * qc + 4

                    def emit_av(j, w_, off_):
                        for h in range(HPC):
                            lhs = vA[b][:, j * 130 + h * 65:
                                        j * 130 + h * 65 + 65]
                            nc.tensor.matmul(av[h][:, off_:QC],
                                             lhs, ework[j][:, h * w_:
                                                           (h + 1) * w_],
                                             start=(j == 0),
                                             stop=(j == nj - 1))

                    ework = {}
                    geom = {}
                    # software-pipelined j-loop: av(j-1) is emitted AFTER
                    # sc(j)/exp(j) so the in-order PE queue overlaps the
                    # ACT exp of tile j with the AV matmul of tile j-1
                    for j in range(nj):
                        jr = j - 4 * qc
                        off = max(jr, 0) * 128
                        w = QC - off
                        geom[j] = (w, off)
                        qsl = slice(qc * QC + off, (qc + 1) * QC)
                        # both heads' scores into one 2-bank psum tile
                        sc = psB.tile([128, 2 * QC], f32, name="scb",
                                      tag="scb", bufs=2)
                        for h in range(HPC):
                            hp = slice(h * 64, (h + 1) * 64)
                            nc.tensor.matmul(
                                sc[:, h * QC:h * QC + w],
                                kT[b][hp, j * 128:(j + 1) * 128],
                                qT[b][hp, qsl], start=True, stop=True)
                        # one exp for both heads via a strided AP
                        e = pB.tile([128, 2 * w], DT, name="exb",
                                    tag="exb", bufs=4)
                        ework[j] = e
                        sc3 = sc[:].rearrange("p (two q) -> p two q",
                                              two=2)[:, :, 0:w]
                        e3 = e[:].rearrange("p (two q) -> p two q", two=2)
                        nc.scalar.activation(e3, sc3, EXP,
                                             scale=1.0 / np.sqrt(HS))
                        if jr >= 0:
                            for h in range(HPC):
                                nc.vector.tensor_mul(
                                    e[:, h * w:h * w + 128],
                                    e[:, h * w:h * w + 128], triu_sb[:])
                        if j == 2:
                            flush_norm()
                        if j >= 1:
                            wp_, op_ = geom[j - 1]
                            emit_av(j - 1, wp_, op_)
                            del ework[j - 1]
                    wl, ol = geom[nj - 1]
                    emit_av(nj - 1, wl, ol)
                    pending[0] = (b, qc, av)

                def proj_group(g, pC):
                    # context tiles stationary (8 ldweights), Wp moving
                    cx = []
                    for j in range(ND):
                        t = pC.tile([128, TB], DT, name=f"cx{j}",
                                    tag=f"cx{j}", bufs=2)
                        nc.sync.dma_start(t[:], a2a_out[g][j])
                        cx.append(t)
                    ops = []
                    for half in range(2):
                        op = psB.tile([128, 512], f32, name=f"op{half}",
                                      tag="scb", bufs=2)
                        for j in range(ND):
                            nc.tensor.matmul(
                                op[:], cx[j][:],
                                wp_sb[j][:, half * 512:(half + 1) * 512],
                                start=(j == 0), stop=(j == ND - 1))
                        ops.append(op)
                    os_ = pC.tile([128, D], f32, name="os", tag="os", bufs=2)
                    for half in range(2):
                        nc.vector.tensor_add(
                            os_[:, half * 512:(half + 1) * 512], ops[half][:],
                            bp_sb[:, half * 512:(half + 1) * 512])
                    nc.sync.dma_start(out_d[g * TB:(g + 1) * TB, :], os_[:])

                with tc.tile_pool(name="pC", bufs=2) as pC:
                    with nc.named_scope("phA"):
                        for b in range(B):
                            phase_a(b)
                    psA_ctx.__exit__(None, None, None)
                    for nm in ("psB", "psAV"):
                        c = tc.tile_pool(name=nm, bufs=2, space="PSUM")
                        _ps_ctxs.append(c)
                    psB = _ps_ctxs[0].__enter__()
                    psAV = _ps_ctxs[1].__enter__()
                    for b in range(B):
                        with nc.named_scope(f"phB{b}"):
                            for qc in range(NQC):
                                attn_chunk(b, qc)
                                if qc == 1 and b >= 1:
                                    with nc.named_scope(f"phC{2*b-1}"):
                                        proj_group(2 * b - 1, pC)
                                if qc == 3:
                                    with nc.named_scope(f"phC{2*b}"):
                                        proj_group(2 * b, pC)
                    flush_norm()
                    with nc.named_scope(f"phC{NG-1}"):
                        proj_group(NG - 1, pC)
                    for c in reversed(_ps_ctxs):
                        c.__exit__(None, None, None)

    nc.compile()
    return nc


def prep_inputs(x, Wq, Wk, Wv, Wp, bp, T, dt_name=DT_NAME):
    """Host-side sharding/layout prep. Returns in_maps for the 8 cores."""
    DT = {"f32r": f32r, "bf16": bf16, "f32": f32}[dt_name]
    ndt = _np_dt(DT)
    BT = B * T
    NTB = T // KT

    x = np.asarray(x, np.float32)
    Wq = np.asarray(Wq, np.float32)
    Wk = np.asarray(Wk, np.float32)
    Wv = np.asarray(Wv, np.float32)
    Wp = np.asarray(Wp, np.float32)
    bp = np.asarray(bp, np.float32)

    xt = np.ascontiguousarray(x.reshape(BT, D).T).astype(ndt)
    wp = np.ascontiguousarray(Wp.T).astype(ndt)
    bpr = np.ascontiguousarray(np.broadcast_to(bp.reshape(1, D), (128, D)))
    triu = np.triu(np.ones((128, 128), np.float32)).astype(ndt)
    ones1 = np.ones((65, 64), np.float32)
    onesm = np.ones((128, NTB), np.float32).astype(ndt)

    def wslice(W, c):
        # [H, D, HS] heads 2c,2c+1 -> [D, 128] as [d, (h_local, e)]
        return np.ascontiguousarray(
            W[2 * c:2 * c + 2].transpose(1, 0, 2).reshape(D, 2 * HS)).astype(ndt)

    in_maps = []
    for c in range(N_CORES):
        in_maps.append({
            "xt": xt, "wq": wslice(Wq, c), "wk": wslice(Wk, c),
            "wv": wslice(Wv, c), "wp": wp, "bp": bpr,
            "triu": triu, "ones1": ones1, "onesm": onesm,
        })
    return in_maps


def gather_out(results, T):
    """results[c]["outT"] is [SL, D] token-major; core c's row block g holds
    token sub-block (b= g//2, l = (g%2)*8 + c) of 128 tokens."""
    BT = B * T
    full = np.empty((BT, D), np.float32)
    for c in range(N_CORES):
        o = np.asarray(results[c]["outT"])
        for g in range(2 * B):
            b, hh = g // 2, g % 2
            t0 = b * T + (hh * 8 + c) * 128
            full[t0:t0 + 128, :] = o[g * 128:(g + 1) * 128, :]
    return full.reshape(B, T, D)


_NC_CACHE = {}


def kernel(x, Wq, Wk, Wv, Wp, bp):
    T = np.asarray(x).shape[1]
    key = (T, DT_NAME)
    if key not in _NC_CACHE:
        _NC_CACHE[key] = build_nc(T, DT_NAME)
    nc = _NC_CACHE[key]
    in_maps = prep_inputs(x, Wq, Wk, Wv, Wp, bp, T, DT_NAME)
    res = run_bass_kernel_spmd(nc, in_maps, list(range(N_CORES)))
    return np.ascontiguousarray(
        gather_out(res.results, T).astype(np.float32))


# revision 15
# speedup vs baseline: 1.0524x; 1.0384x over previous
"""Multi-head causal attention (B=4, T=2048, D=1024, H=16, HS=64) on 8 TRN2 cores.

Sharding: tensor-parallel over heads (2 heads/core) for QKV+attention, then
AllToAlls redistribute per-head context to token-parallel layout for the output
projection. Two AllToAlls per batch (8 total, 128-token sub-blocks), so the
projection of one half-batch overlaps the attention of the next.

Engine assignment discipline (v1 was ACT-bound, v2 was LDWEIGHTS-bound):
  - ACT (scalar) runs ONLY the softmax exp -> single activation-table load.
  - All PSUM->SBUF copies and the context normalization run on DVE.
  - Softmax reciprocal: DVE reciprocal_approx_fast on the PSUM denominator row
    (partition 64), rounded to f32r, partition-broadcast by a K=1 PE matmul
    whose stationary ones-row also lives on partition 64 (tile_position 64,0).
  - V is computed transpose-free (x^T t-tile stationary, Wv moving -> [token,
    head*hs]); its 32 small weight-loads per chunk hide under the q/k N=512
    streams by interleaving the matmuls.
  - Output projection keeps the received context STATIONARY (8 weight-loads
    per group) and streams Wp; output is token-major so no transpose.
  - Normalization of chunk c is emitted inside chunk c+1's first j-tile so
    the PE/ACT queues never drain at chunk boundaries.

All matmuls bf16 with fp32 PSUM accumulation; softmax without max-subtraction
(|scores| <= ~8 for these inputs, exp is safe in fp32).
"""
import numpy as np

import concourse.bass as bass
import concourse.tile as tile
from concourse import bacc, mybir
from concourse.bass_utils import run_bass_kernel_spmd

f32 = mybir.dt.float32
f32r = mybir.dt.float32r
bf16 = mybir.dt.bfloat16

B, D, H, HS = 4, 1024, 16, 64
N_CORES = 8
HPC = H // N_CORES          # heads per core
QC = 512                    # q-chunk width
KT = 128                    # k-tile width
ND = D // 128               # din tiles

DT_NAME = "bf16"            # "f32r" | "bf16" | "f32"


def _np_dt(dt):
    import ml_dtypes
    return {f32: np.float32, f32r: np.float32, bf16: ml_dtypes.bfloat16}[dt]


def build_nc(T=2048, dt_name=DT_NAME):
    DT = {"f32r": f32r, "bf16": bf16, "f32": f32}[dt_name]
    BT = B * T
    SL = BT // N_CORES              # tokens per core in phase C (1024)
    NQC = T // QC                   # q-chunks per batch (4)
    NTB = T // KT                   # k-tiles per batch (16)
    NG = 2 * B                      # two AllToAlls per batch
    TB = 128                        # token sub-block (a2a slot width)
    assert NQC == 4 and SL == NG * TB

    nc = bacc.Bacc("TRN2", target_bir_lowering=False, debug=False,
                   num_devices=N_CORES)

    xt_d = nc.dram_tensor("xt", [D, BT], DT, kind="ExternalInput").ap()
    wq_d = nc.dram_tensor("wq", [D, 128], DT, kind="ExternalInput").ap()
    wk_d = nc.dram_tensor("wk", [D, 128], DT, kind="ExternalInput").ap()
    wv_d = nc.dram_tensor("wv", [D, 128], DT, kind="ExternalInput").ap()
    wp_d = nc.dram_tensor("wp", [D, D], DT, kind="ExternalInput").ap()
    bp_d = nc.dram_tensor("bp", [128, D], f32, kind="ExternalInput").ap()
    tril_d = nc.dram_tensor("triu", [128, 128], DT, kind="ExternalInput").ap()
    ones1_d = nc.dram_tensor("ones1", [65, 64], f32r, kind="ExternalInput").ap()
    onesm_d = nc.dram_tensor("onesm", [128, NTB], DT, kind="ExternalInput").ap()
    out_d = nc.dram_tensor("outT", [SL, D], f32, kind="ExternalOutput").ap()

    EXP = mybir.ActivationFunctionType.Exp

    with tile.TileContext(nc) as tc:
        with (
            tc.tile_pool(name="wts", bufs=1) as wts,
            tc.tile_pool(name="acts", bufs=1) as acts,
            tc.tile_pool(name="dram", bufs=1, space="DRAM") as dram,
        ):
            # ---- persistent loads ----
            wq_sb, wk_sb, wv_sb = [], [], []
            for j in range(ND):
                for lst, dd, nm in ((wq_sb, wq_d, "wq"), (wk_sb, wk_d, "wk"),
                                    (wv_sb, wv_d, "wv")):
                    t = wts.tile([128, 128], DT, name=f"{nm}{j}", tag=f"{nm}{j}")
                    nc.sync.dma_start(t[:], dd[j * 128:(j + 1) * 128, :])
                    lst.append(t)
            triu_sb = wts.tile([128, 128], DT, name="triu", tag="triu")
            nc.sync.dma_start(triu_sb[:], tril_d[:])
            ones1_sb = wts.tile([65, 64], f32r, name="ones1", tag="ones1")
            nc.sync.dma_start(ones1_sb[:], ones1_d[:])
            onesm_sb = wts.tile([128, NTB], DT, name="onesm", tag="onesm")
            nc.sync.dma_start(onesm_sb[:], onesm_d[:])

            a2a_in = [dram.tile([N_CORES, 128, TB], DT, name=f"a2ai{g}")
                      for g in range(NG)]
            a2a_out = [dram.tile([N_CORES, 128, TB], DT, name=f"a2ao{g}")
                       for g in range(NG)]

            # per-batch activation tensors
            qT, kT, vA = [], [], []
            for b in range(B):
                qT.append(acts.tile([128, T], DT, name=f"qT{b}", tag=f"qT{b}"))
                kT.append(acts.tile([128, T], DT, name=f"kT{b}", tag=f"kT{b}"))
                vA.append(acts.tile([128, NTB * 130], DT, name=f"vA{b}",
                                    tag=f"vA{b}"))

            # output-projection weights load late so phase A's x-stream gets
            # the DMA queues first
            wp_sb = []
            for j in range(ND):
                t = wts.tile([128, D], DT, name=f"wp{j}", tag=f"wp{j}")
                nc.sync.dma_start(t[:], wp_d[j * 128:(j + 1) * 128, :])
                wp_sb.append(t)
            bp_sb = wts.tile([128, D], f32, name="bpr", tag="bpr")
            nc.sync.dma_start(bp_sb[:], bp_d[:])

            with (
                tc.tile_pool(name="pA", bufs=2) as pA,
                tc.tile_pool(name="pB", bufs=3) as pB,
            ):
                psB_ctx = tc.tile_pool(name="psB", bufs=2, space="PSUM")
                psAV_ctx = tc.tile_pool(name="psAV", bufs=2, space="PSUM")
                psB = psB_ctx.__enter__()
                psAV = psAV_ctx.__enter__()

                def phase_a_chunk(b, ch):
                    """QKV projection for one 512-token chunk. Emitted as PE
                    filler inside the previous batch's (ACT-bound) attention
                    so the tensor engine stays dense and HAM-warm."""
                    if ch == 0:
                        # ones columns of the augmented-V slots ([v0|1|v1|1])
                        v3 = vA[b][:].rearrange("p (t c) -> p t c", c=130)
                        nc.vector.tensor_copy(v3[:, :, 64], onesm_sb[:])
                        nc.vector.tensor_copy(v3[:, :, 129], onesm_sb[:])
                    i0 = b * T + ch * QC
                    # one 3D DMA for all 8 din-tiles of the chunk
                    xtl = pA.tile([128, ND * QC], DT, name="xtl",
                                  tag="xtl", bufs=2)
                    nc.sync.dma_start(
                        xtl[:].rearrange("p (j c) -> p j c", j=ND),
                        xt_d[:, i0:i0 + QC].rearrange(
                            "(j p) c -> p j c", p=128))
                    xt = [xtl[:, j * QC:(j + 1) * QC] for j in range(ND)]
                    sl = slice(ch * QC, (ch + 1) * QC)
                    for w_sb, dst in ((wq_sb, qT[b]), (wk_sb, kT[b])):
                        pp = psB.tile([128, QC], f32, name="pp", tag="scb",
                                      bufs=2)
                        for j in range(ND):
                            nc.tensor.matmul(pp[:], w_sb[j][:], xt[j],
                                             start=(j == 0),
                                             stop=(j == ND - 1))
                        nc.vector.tensor_copy(dst[:, sl], pp[:])
                    for tt in range(4):
                        vp = psB.tile([128, 128], f32, name=f"vp{tt}",
                                      tag="scb", bufs=2)
                        for j in range(ND):
                            nc.tensor.matmul(
                                vp[:], xt[j][:, tt * 128:(tt + 1) * 128],
                                wv_sb[j][:], start=(j == 0),
                                stop=(j == ND - 1))
                        slot = (ch * 4 + tt) * 130
                        dst3 = vA[b][:, slot:slot + 130].rearrange(
                            "p (s c) -> p s c", c=65)[:, :, 0:64]
                        src3 = vp[:].rearrange("p (s c) -> p s c", c=64)
                        nc.vector.tensor_copy(dst3, src3)

                pending = [None]

                def flush_norm():
                    if pending[0] is None:
                        return
                    pb_, pqc, pav = pending[0]
                    pending[0] = None
                    for h in range(HPC):
                        # hop the PSUM denominator row (partition 64) to
                        # partition 0 with a plain DVE copy (HW-proven);
                        # reciprocal_approx_fast requires base partition 0
                        den = pB.tile([1, QC], f32, name=f"den{h}",
                                      tag=f"den{h}", bufs=2)
                        nc.vector.tensor_copy(den[:], pav[h][64:65, :])
                        rec = pB.tile([1, QC], f32, name=f"rec{h}",
                                      tag=f"rec{h}", bufs=2)
                        nc.vector.reciprocal_approx_fast(rec[:], den[:])
                        recr = pB.tile([1, QC], f32r, name=f"recr{h}",
                                       tag=f"recr{h}", bufs=2)
                        nc.vector.tensor_copy(recr[:], rec[:])
                        bcp = psB.tile([64, QC], f32, name=f"bcp{h}",
                                       tag="scb", bufs=2)
                        nc.tensor.matmul(bcp[:], ones1_sb[0:1, :],
                                         recr[:],
                                         start=True, stop=True)
                        bcs = pB.tile([64, QC], f32, name=f"bcs{h}",
                                      tag=f"bcs{h}", bufs=2)
                        nc.vector.tensor_copy(bcs[:], bcp[:])
                        ctx = pB.tile([64, QC], DT, name=f"ctx{h}",
                                      tag=f"ctx{h}", bufs=2)
                        nc.vector.tensor_mul(ctx[:], pav[h][0:64, :], bcs[:])
                        # 4 token sub-blocks to their a2a slots
                        for lt in range(4):
                            l = 4 * pqc + lt
                            g = 2 * pb_ + l // 8
                            nc.sync.dma_start(
                                a2a_in[g][l % 8, h * 64:(h + 1) * 64, :],
                                ctx[:, lt * TB:(lt + 1) * TB])
                    if pqc % 2 == 1:
                        g = 2 * pb_ + pqc // 2
                        nc.gpsimd.collective_compute(
                            "AllToAll", mybir.AluOpType.bypass,
                            replica_groups=[list(range(N_CORES))],
                            ins=[a2a_in[g].opt()], outs=[a2a_out[g].opt()])

                def attn_chunk(b, qc):
                    av = [psAV.tile([65, QC], f32, name=f"av{h}",
                                    tag=f"av{h}", bufs=2)
                          for h in range(HPC)]
                    nj = 4 * qc + 4

                    def emit_av(j, w_, off_):
                        for h in range(HPC):
                            lhs = vA[b][:, j * 130 + h * 65:
                                        j * 130 + h * 65 + 65]
                            nc.tensor.matmul(av[h][:, off_:QC],
                                             lhs, ework[j][:, h * w_:
                                                           (h + 1) * w_],
                                             start=(j == 0),
                                             stop=(j == nj - 1))

                    ework = {}
                    geom = {}
                    # software-pipelined j-loop: av(j-1) is emitted AFTER
                    # sc(j)/exp(j) so the in-order PE queue overlaps the
                    # ACT exp of tile j with the AV matmul of tile j-1
                    for j in range(nj):
                        jr = j - 4 * qc
                        off = max(jr, 0) * 128
                        w = QC - off
                        geom[j] = (w, off)
                        qsl = slice(qc * QC + off, (qc + 1) * QC)
                        # both heads' scores into one 2-bank psum tile
                        sc = psB.tile([128, 2 * QC], f32, name="scb",
                                      tag="scb", bufs=2)
                        for h in range(HPC):
                            hp = slice(h * 64, (h + 1) * 64)
                            nc.tensor.matmul(
                                sc[:, h * QC:h * QC + w],
                                kT[b][hp, j * 128:(j + 1) * 128],
                                qT[b][hp, qsl], start=True, stop=True)
                        # one exp for both heads via a strided AP
                        e = pB.tile([128, 2 * w], DT, name="exb",
                                    tag="exb", bufs=4)
                        ework[j] = e
                        sc3 = sc[:].rearrange("p (two q) -> p two q",
                                              two=2)[:, :, 0:w]
                        e3 = e[:].rearrange("p (two q) -> p two q", two=2)
                        nc.scalar.activation(e3, sc3, EXP,
                                             scale=1.0 / np.sqrt(HS))
                        if jr >= 0:
                            for h in range(HPC):
                                nc.vector.tensor_mul(
                                    e[:, h * w:h * w + 128],
                                    e[:, h * w:h * w + 128], triu_sb[:])
                        if j == 2:
                            flush_norm()
                        if j >= 1:
                            wp_, op_ = geom[j - 1]
                            emit_av(j - 1, wp_, op_)
                            del ework[j - 1]
                    wl, ol = geom[nj - 1]
                    emit_av(nj - 1, wl, ol)
                    pending[0] = (b, qc, av)

                def proj_group(g, pC):
                    # context tiles stationary (8 ldweights), Wp moving
                    cx = []
                    for j in range(ND):
                        t = pC.tile([128, TB], DT, name=f"cx{j}",
                                    tag=f"cx{j}", bufs=2)
                        nc.sync.dma_start(t[:], a2a_out[g][j])
                        cx.append(t)
                    ops = []
                    for half in range(2):
                        op = psB.tile([128, 512], f32, name=f"op{half}",
                                      tag="scb", bufs=2)
                        for j in range(ND):
                            nc.tensor.matmul(
                                op[:], cx[j][:],
                                wp_sb[j][:, half * 512:(half + 1) * 512],
                                start=(j == 0), stop=(j == ND - 1))
                        ops.append(op)
                    os_ = pC.tile([128, D], f32, name="os", tag="os", bufs=2)
                    for half in range(2):
                        nc.vector.tensor_add(
                            os_[:, half * 512:(half + 1) * 512], ops[half][:],
                            bp_sb[:, half * 512:(half + 1) * 512])
                    nc.sync.dma_start(out_d[g * TB:(g + 1) * TB, :], os_[:])

                with tc.tile_pool(name="pC", bufs=2) as pC:
                    with nc.named_scope("phA0"):
                        for ch in range(NQC):
                            phase_a_chunk(0, ch)
                    for b in range(B):
                        with nc.named_scope(f"phB{b}"):
                            for qc in range(NQC):
                                attn_chunk(b, qc)
                                if qc == 1 and b >= 1:
                                    with nc.named_scope(f"phC{2*b-1}"):
                                        proj_group(2 * b - 1, pC)
                                if qc == 3:
                                    with nc.named_scope(f"phC{2*b}"):
                                        proj_group(2 * b, pC)
                                if b + 1 < B:
                                    with nc.named_scope(f"phA{b+1}"):
                                        phase_a_chunk(b + 1, qc)
                    flush_norm()
                    with nc.named_scope(f"phC{NG-1}"):
                        proj_group(NG - 1, pC)
                    psAV_ctx.__exit__(None, None, None)
                    psB_ctx.__exit__(None, None, None)

    nc.compile()
    return nc


def prep_inputs(x, Wq, Wk, Wv, Wp, bp, T, dt_name=DT_NAME):
    """Host-side sharding/layout prep. Returns in_maps for the 8 cores."""
    DT = {"f32r": f32r, "bf16": bf16, "f32": f32}[dt_name]
    ndt = _np_dt(DT)
    BT = B * T
    NTB = T // KT

    x = np.asarray(x, np.float32)
    Wq = np.asarray(Wq, np.float32)
    Wk = np.asarray(Wk, np.float32)
    Wv = np.asarray(Wv, np.float32)
    Wp = np.asarray(Wp, np.float32)
    bp = np.asarray(bp, np.float32)

    xt = np.ascontiguousarray(x.reshape(BT, D).T).astype(ndt)
    wp = np.ascontiguousarray(Wp.T).astype(ndt)
    bpr = np.ascontiguousarray(np.broadcast_to(bp.reshape(1, D), (128, D)))
    triu = np.triu(np.ones((128, 128), np.float32)).astype(ndt)
    ones1 = np.ones((65, 64), np.float32)
    onesm = np.ones((128, NTB), np.float32).astype(ndt)

    def wslice(W, c):
        # [H, D, HS] heads 2c,2c+1 -> [D, 128] as [d, (h_local, e)]
        return np.ascontiguousarray(
            W[2 * c:2 * c + 2].transpose(1, 0, 2).reshape(D, 2 * HS)).astype(ndt)

    in_maps = []
    for c in range(N_CORES):
        in_maps.append({
            "xt": xt, "wq": wslice(Wq, c), "wk": wslice(Wk, c),
            "wv": wslice(Wv, c), "wp": wp, "bp": bpr,
            "triu": triu, "ones1": ones1, "onesm": onesm,
        })
    return in_maps


def gather_out(results, T):
    """results[c]["outT"] is [SL, D] token-major; core c's row block g holds
    token sub-block (b= g//2, l = (g%2)*8 + c) of 128 tokens."""
    BT = B * T
    full = np.empty((BT, D), np.float32)
    for c in range(N_CORES):
        o = np.asarray(results[c]["outT"])
        for g in range(2 * B):
            b, hh = g // 2, g % 2
            t0 = b * T + (hh * 8 + c) * 128
            full[t0:t0 + 128, :] = o[g * 128:(g + 1) * 128, :]
    return full.reshape(B, T, D)


_NC_CACHE = {}


def kernel(x, Wq, Wk, Wv, Wp, bp):
    T = np.asarray(x).shape[1]
    key = (T, DT_NAME)
    if key not in _NC_CACHE:
        _NC_CACHE[key] = build_nc(T, DT_NAME)
    nc = _NC_CACHE[key]
    in_maps = prep_inputs(x, Wq, Wk, Wv, Wp, bp, T, DT_NAME)
    res = run_bass_kernel_spmd(nc, in_maps, list(range(N_CORES)))
    return np.ascontiguousarray(
        gather_out(res.results, T).astype(np.float32))


# revision 20
# speedup vs baseline: 1.0699x; 1.0167x over previous
"""Multi-head causal attention (B=4, T=2048, D=1024, H=16, HS=64) on 8 TRN2 cores.

Sharding: tensor-parallel over heads (2 heads/core) for QKV+attention, then
AllToAlls redistribute per-head context to token-parallel layout for the output
projection. Two AllToAlls per batch (8 total, 128-token sub-blocks), so the
projection of one half-batch overlaps the attention of the next.

Engine assignment discipline (v1 was ACT-bound, v2 was LDWEIGHTS-bound):
  - ACT (scalar) runs ONLY the softmax exp -> single activation-table load.
  - All PSUM->SBUF copies and the context normalization run on DVE.
  - Softmax reciprocal: DVE reciprocal_approx_fast on the PSUM denominator row
    (partition 64), rounded to f32r, partition-broadcast by a K=1 PE matmul
    whose stationary ones-row also lives on partition 64 (tile_position 64,0).
  - V is computed transpose-free (x^T t-tile stationary, Wv moving -> [token,
    head*hs]); its 32 small weight-loads per chunk hide under the q/k N=512
    streams by interleaving the matmuls.
  - Output projection keeps the received context STATIONARY (8 weight-loads
    per group) and streams Wp; output is token-major so no transpose.
  - Normalization of chunk c is emitted inside chunk c+1's first j-tile so
    the PE/ACT queues never drain at chunk boundaries.

All matmuls bf16 with fp32 PSUM accumulation; softmax without max-subtraction
(|scores| <= ~8 for these inputs, exp is safe in fp32).
"""
import numpy as np

import concourse.bass as bass
import concourse.tile as tile
from concourse import bacc, mybir
from concourse.bass_utils import run_bass_kernel_spmd

f32 = mybir.dt.float32
f32r = mybir.dt.float32r
bf16 = mybir.dt.bfloat16

B, D, H, HS = 4, 1024, 16, 64
N_CORES = 8
HPC = H // N_CORES          # heads per core
QC = 512                    # q-chunk width
KT = 128                    # k-tile width
ND = D // 128               # din tiles

DT_NAME = "bf16"            # "f32r" | "bf16" | "f32"


def _np_dt(dt):
    import ml_dtypes
    return {f32: np.float32, f32r: np.float32, bf16: ml_dtypes.bfloat16}[dt]


def build_nc(T=2048, dt_name=DT_NAME):
    DT = {"f32r": f32r, "bf16": bf16, "f32": f32}[dt_name]
    BT = B * T
    SL = BT // N_CORES              # tokens per core in phase C (1024)
    NQC = T // QC                   # q-chunks per batch (4)
    NTB = T // KT                   # k-tiles per batch (16)
    NG = 2 * B                      # two AllToAlls per batch
    TB = 128                        # token sub-block (a2a slot width)
    assert NQC == 4 and SL == NG * TB

    nc = bacc.Bacc("TRN2", target_bir_lowering=False, debug=False,
                   num_devices=N_CORES)

    xt_d = nc.dram_tensor("xt", [D, BT], DT, kind="ExternalInput").ap()
    wq_d = nc.dram_tensor("wq", [D, 128], DT, kind="ExternalInput").ap()
    wk_d = nc.dram_tensor("wk", [D, 128], DT, kind="ExternalInput").ap()
    wv_d = nc.dram_tensor("wv", [D, 128], DT, kind="ExternalInput").ap()
    wp_d = nc.dram_tensor("wp", [D, D], DT, kind="ExternalInput").ap()
    bp_d = nc.dram_tensor("bp", [128, D], f32, kind="ExternalInput").ap()
    tril_d = nc.dram_tensor("triu", [128, 128], DT, kind="ExternalInput").ap()
    ones1_d = nc.dram_tensor("ones1", [65, 64], f32r, kind="ExternalInput").ap()
    onesm_d = nc.dram_tensor("onesm", [128, NTB], DT, kind="ExternalInput").ap()
    out_d = nc.dram_tensor("outT", [SL, D], f32, kind="ExternalOutput").ap()

    EXP = mybir.ActivationFunctionType.Exp

    with tile.TileContext(nc) as tc:
        with (
            tc.tile_pool(name="wts", bufs=1) as wts,
            tc.tile_pool(name="acts", bufs=1) as acts,
            tc.tile_pool(name="dram", bufs=1, space="DRAM") as dram,
        ):
            # ---- persistent loads (one batched DMA per weight tensor) ----
            wq_sb, wk_sb, wv_sb = [], [], []
            for lst, dd, nm in ((wq_sb, wq_d, "wq"), (wk_sb, wk_d, "wk"),
                                (wv_sb, wv_d, "wv")):
                big = wts.tile([128, ND * 128], DT, name=f"{nm}L",
                               tag=f"{nm}L")
                nc.sync.dma_start(
                    big[:].rearrange("p (j e) -> p j e", j=ND),
                    dd[:].rearrange("(j p) e -> p j e", p=128))
                for j in range(ND):
                    lst.append(big[:, j * 128:(j + 1) * 128])
            triu_sb = wts.tile([128, 128], DT, name="triu", tag="triu")
            nc.sync.dma_start(triu_sb[:], tril_d[:])
            ones1_sb = wts.tile([65, 64], f32r, name="ones1", tag="ones1")
            nc.sync.dma_start(ones1_sb[:], ones1_d[:])
            onesm_sb = wts.tile([128, NTB], DT, name="onesm", tag="onesm")
            nc.sync.dma_start(onesm_sb[:], onesm_d[:])

            a2a_in = [dram.tile([N_CORES, 128, TB], DT, name=f"a2ai{g}")
                      for g in range(NG)]
            a2a_out = [dram.tile([N_CORES, 128, TB], DT, name=f"a2ao{g}")
                       for g in range(NG)]

            # per-batch activation tensors
            qT, kT, vA = [], [], []
            for b in range(B):
                qT.append(acts.tile([128, T], DT, name=f"qT{b}", tag=f"qT{b}"))
                kT.append(acts.tile([128, T], DT, name=f"kT{b}", tag=f"kT{b}"))
                vA.append(acts.tile([128, NTB * 130], DT, name=f"vA{b}",
                                    tag=f"vA{b}"))

            # output-projection weights: tiles allocated here, DMAs emitted
            # late (after batch 0's x-stream) in the main loop below
            wpL = wts.tile([128, ND * D], DT, name="wpL", tag="wpL")
            wp_sb = [wpL[:, j * D:(j + 1) * D] for j in range(ND)]
            bp_sb = wts.tile([128, D], f32, name="bpr", tag="bpr")

            with (
                tc.tile_pool(name="pA", bufs=2) as pA,
                tc.tile_pool(name="pB", bufs=3) as pB,
            ):
                psB_ctx = tc.tile_pool(name="psB", bufs=2, space="PSUM")
                psAV_ctx = tc.tile_pool(name="psAV", bufs=2, space="PSUM")
                psB = psB_ctx.__enter__()
                psAV = psAV_ctx.__enter__()

                # ---- fine-grained phase-A filler units ----
                # Each unit is one full accumulate+copy (no PSUM slot held
                # across other allocations). Units are pulled between the
                # attention j-tiles so the PE stays dense (HAM-warm) while
                # ACT grinds exps, without starving either queue.
                filler_q = []

                def enqueue_a_chunk(b, ch):
                    if ch == 0:
                        # ones columns of the augmented-V slots ([v0|1|v1|1])
                        v3 = vA[b][:].rearrange("p (t c) -> p t c", c=130)
                        nc.vector.tensor_copy(v3[:, :, 64], onesm_sb[:])
                        nc.vector.tensor_copy(v3[:, :, 129], onesm_sb[:])
                    i0 = b * T + ch * QC
                    # one 3D DMA for all 8 din-tiles of the chunk
                    xtl = pA.tile([128, ND * QC], DT, name="xtl",
                                  tag="xtl", bufs=3)
                    nc.sync.dma_start(
                        xtl[:].rearrange("p (j c) -> p j c", j=ND),
                        xt_d[:, i0:i0 + QC].rearrange(
                            "(j p) c -> p j c", p=128))
                    xt = [xtl[:, j * QC:(j + 1) * QC] for j in range(ND)]
                    sl = slice(ch * QC, (ch + 1) * QC)

                    def qk_unit(w_sb, dst):
                        def u():
                            pp = psB.tile([128, QC], f32, name="ppf",
                                          tag="afill", bufs=2)
                            for j in range(ND):
                                nc.tensor.matmul(pp[:], w_sb[j][:], xt[j],
                                                 start=(j == 0),
                                                 stop=(j == ND - 1))
                            nc.vector.tensor_copy(dst[:, sl], pp[:])
                        return u

                    def v_unit(tt):
                        def u():
                            vp = psB.tile([128, 128], f32, name="vpf",
                                          tag="afill", bufs=2)
                            for j in range(ND):
                                nc.tensor.matmul(
                                    vp[:], xt[j][:, tt * 128:(tt + 1) * 128],
                                    wv_sb[j][:], start=(j == 0),
                                    stop=(j == ND - 1))
                            slot = (ch * 4 + tt) * 130
                            dst3 = vA[b][:, slot:slot + 130].rearrange(
                                "p (s c) -> p s c", c=65)[:, :, 0:64]
                            src3 = vp[:].rearrange("p (s c) -> p s c", c=64)
                            nc.vector.tensor_copy(dst3, src3)
                        return u

                    key = (b, ch)
                    filler_q.append((key, qk_unit(wq_sb, qT[b])))
                    filler_q.append((key, qk_unit(wk_sb, kT[b])))
                    for tt in range(4):
                        filler_q.append((key, v_unit(tt)))

                def pull_filler(n=1):
                    for _ in range(n):
                        if not filler_q:
                            return
                        filler_q.pop(0)[1]()

                def drain_fillers_for(b, qc):
                    while filler_q and filler_q[0][0] <= (b, qc):
                        filler_q.pop(0)[1]()

                def flush_norm(pb_, pqc, pav):
                    for h in range(HPC):
                        # hop the PSUM denominator row (partition 64) to
                        # partition 0 with a plain DVE copy (HW-proven);
                        # reciprocal_approx_fast requires base partition 0
                        den = pB.tile([1, QC], f32, name=f"den{h}",
                                      tag=f"den{h}", bufs=2)
                        nc.vector.tensor_copy(den[:], pav[h][64:65, :])
                        rec = pB.tile([1, QC], f32, name=f"rec{h}",
                                      tag=f"rec{h}", bufs=2)
                        nc.vector.reciprocal_approx_fast(rec[:], den[:])
                        recr = pB.tile([1, QC], f32r, name=f"recr{h}",
                                       tag=f"recr{h}", bufs=2)
                        nc.vector.tensor_copy(recr[:], rec[:])
                        bcp = psB.tile([64, QC], f32, name=f"bcp{h}",
                                       tag="scb", bufs=2)
                        nc.tensor.matmul(bcp[:], ones1_sb[0:1, :],
                                         recr[:],
                                         start=True, stop=True)
                        bcs = pB.tile([64, QC], f32, name=f"bcs{h}",
                                      tag=f"bcs{h}", bufs=2)
                        nc.vector.tensor_copy(bcs[:], bcp[:])
                        ctx = pB.tile([64, QC], DT, name=f"ctx{h}",
                                      tag=f"ctx{h}", bufs=2)
                        nc.vector.tensor_mul(ctx[:], pav[h][0:64, :], bcs[:])
                        # 4 token sub-blocks to their a2a slots
                        for lt in range(4):
                            l = 4 * pqc + lt
                            g = 2 * pb_ + l // 8
                            nc.sync.dma_start(
                                a2a_in[g][l % 8, h * 64:(h + 1) * 64, :],
                                ctx[:, lt * TB:(lt + 1) * TB])
                    if pqc % 2 == 1:
                        g = 2 * pb_ + pqc // 2
                        nc.gpsimd.collective_compute(
                            "AllToAll", mybir.AluOpType.bypass,
                            replica_groups=[list(range(N_CORES))],
                            ins=[a2a_in[g].opt()], outs=[a2a_out[g].opt()])

                def attn_chunk(b, qc):
                    drain_fillers_for(b, qc)
                    if b + 1 < B:
                        enqueue_a_chunk(b + 1, qc)
                    av = [psAV.tile([65, QC], f32, name=f"av{h}",
                                    tag=f"av{h}", bufs=1)
                          for h in range(HPC)]
                    nj = 4 * qc + 4

                    def emit_av(j, w_, off_):
                        for h in range(HPC):
                            lhs = vA[b][:, j * 130 + h * 65:
                                        j * 130 + h * 65 + 65]
                            nc.tensor.matmul(av[h][:, off_:QC],
                                             lhs, ework[j][:, h * w_:
                                                           (h + 1) * w_],
                                             start=(j == 0),
                                             stop=(j == nj - 1))

                    ework = {}
                    geom = {}
                    # software-pipelined j-loop: av(j-1) is emitted AFTER
                    # sc(j)/exp(j) so the in-order PE queue overlaps the
                    # ACT exp of tile j with the AV matmul of tile j-1
                    for j in range(nj):
                        jr = j - 4 * qc
                        off = max(jr, 0) * 128
                        w = QC - off
                        geom[j] = (w, off)
                        qsl = slice(qc * QC + off, (qc + 1) * QC)
                        # both heads' scores into one 2-bank psum tile
                        sc = psB.tile([128, 2 * QC], f32, name="scb",
                                      tag="scb", bufs=2)
                        for h in range(HPC):
                            hp = slice(h * 64, (h + 1) * 64)
                            nc.tensor.matmul(
                                sc[:, h * QC:h * QC + w],
                                kT[b][hp, j * 128:(j + 1) * 128],
                                qT[b][hp, qsl], start=True, stop=True)
                        # one exp for both heads via a strided AP
                        e = pB.tile([128, 2 * w], DT, name="exb",
                                    tag="exb", bufs=4)
                        ework[j] = e
                        sc3 = sc[:].rearrange("p (two q) -> p two q",
                                              two=2)[:, :, 0:w]
                        e3 = e[:].rearrange("p (two q) -> p two q", two=2)
                        nc.scalar.activation(e3, sc3, EXP,
                                             scale=1.0 / np.sqrt(HS))
                        if jr >= 0:
                            for h in range(HPC):
                                nc.vector.tensor_mul(
                                    e[:, h * w:h * w + 128],
                                    e[:, h * w:h * w + 128], triu_sb[:])
                        if j >= 1:
                            wp_, op_ = geom[j - 1]
                            emit_av(j - 1, wp_, op_)
                            del ework[j - 1]
                            if j % 2 == 1:
                                pull_filler(1)
                    wl, ol = geom[nj - 1]
                    emit_av(nj - 1, wl, ol)
                    # cover the reciprocal-chain latency with filler work
                    pull_filler(2)
                    flush_norm(b, qc, av)

                def proj_group(g, pC):
                    # context tiles stationary (8 ldweights), Wp moving
                    cx = []
                    for j in range(ND):
                        t = pC.tile([128, TB], DT, name=f"cx{j}",
                                    tag=f"cx{j}", bufs=2)
                        nc.sync.dma_start(t[:], a2a_out[g][j])
                        cx.append(t)
                    ops = []
                    for half in range(2):
                        op = psB.tile([128, 512], f32, name=f"op{half}",
                                      tag="scb", bufs=2)
                        for j in range(ND):
                            nc.tensor.matmul(
                                op[:], cx[j][:],
                                wp_sb[j][:, half * 512:(half + 1) * 512],
                                start=(j == 0), stop=(j == ND - 1))
                        ops.append(op)
                    os_ = pC.tile([128, D], f32, name="os", tag="os", bufs=2)
                    for half in range(2):
                        nc.vector.tensor_add(
                            os_[:, half * 512:(half + 1) * 512], ops[half][:],
                            bp_sb[:, half * 512:(half + 1) * 512])
                    nc.sync.dma_start(out_d[g * TB:(g + 1) * TB, :], os_[:])

                with tc.tile_pool(name="pC", bufs=2) as pC:
                    with nc.named_scope("phA0"):
                        for ch in range(NQC):
                            enqueue_a_chunk(0, ch)
                            pull_filler(6)
                    # wp/bp load AFTER batch 0's x-stream owns the queues
                    nc.sync.dma_start(
                        wpL[:].rearrange("p (j c) -> p j c", j=ND),
                        wp_d[:].rearrange("(j p) c -> p j c", p=128))
                    nc.sync.dma_start(bp_sb[:], bp_d[:])
                    for b in range(B):
                        with nc.named_scope(f"phB{b}"):
                            for qc in range(NQC):
                                attn_chunk(b, qc)
                                if qc == 1 and b >= 1:
                                    with nc.named_scope(f"phC{2*b-1}"):
                                        proj_group(2 * b - 1, pC)
                                if qc == 3:
                                    with nc.named_scope(f"phC{2*b}"):
                                        proj_group(2 * b, pC)
                    with nc.named_scope(f"phC{NG-1}"):
                        proj_group(NG - 1, pC)
                    psAV_ctx.__exit__(None, None, None)
                    psB_ctx.__exit__(None, None, None)

    nc.compile()
    return nc


def prep_inputs(x, Wq, Wk, Wv, Wp, bp, T, dt_name=DT_NAME):
    """Host-side sharding/layout prep. Returns in_maps for the 8 cores."""
    DT = {"f32r": f32r, "bf16": bf16, "f32": f32}[dt_name]
    ndt = _np_dt(DT)
    BT = B * T
    NTB = T // KT

    x = np.asarray(x, np.float32)
    Wq = np.asarray(Wq, np.float32)
    Wk = np.asarray(Wk, np.float32)
    Wv = np.asarray(Wv, np.float32)
    Wp = np.asarray(Wp, np.float32)
    bp = np.asarray(bp, np.float32)

    xt = np.ascontiguousarray(x.reshape(BT, D).T).astype(ndt)
    wp = np.ascontiguousarray(Wp.T).astype(ndt)
    bpr = np.ascontiguousarray(np.broadcast_to(bp.reshape(1, D), (128, D)))
    triu = np.triu(np.ones((128, 128), np.float32)).astype(ndt)
    ones1 = np.ones((65, 64), np.float32)
    onesm = np.ones((128, NTB), np.float32).astype(ndt)

    def wslice(W, c):
        # [H, D, HS] heads 2c,2c+1 -> [D, 128] as [d, (h_local, e)]
        return np.ascontiguousarray(
            W[2 * c:2 * c + 2].transpose(1, 0, 2).reshape(D, 2 * HS)).astype(ndt)

    in_maps = []
    for c in range(N_CORES):
        in_maps.append({
            "xt": xt, "wq": wslice(Wq, c), "wk": wslice(Wk, c),
            "wv": wslice(Wv, c), "wp": wp, "bp": bpr,
            "triu": triu, "ones1": ones1, "onesm": onesm,
        })
    return in_maps


def gather_out(results, T):
    """results[c]["outT"] is [SL, D] token-major; core c's row block g holds
    token sub-block (b= g//2, l = (g%2)*8 + c) of 128 tokens."""
    BT = B * T
    full = np.empty((BT, D), np.float32)
    for c in range(N_CORES):
        o = np.asarray(results[c]["outT"])
        for g in range(2 * B):
            b, hh = g // 2, g % 2
            t0 = b * T + (hh * 8 + c) * 128
            full[t0:t0 + 128, :] = o[g * 128:(g + 1) * 128, :]
    return full.reshape(B, T, D)


_NC_CACHE = {}


def kernel(x, Wq, Wk, Wv, Wp, bp):
    T = np.asarray(x).shape[1]
    key = (T, DT_NAME)
    if key not in _NC_CACHE:
        _NC_CACHE[key] = build_nc(T, DT_NAME)
    nc = _NC_CACHE[key]
    in_maps = prep_inputs(x, Wq, Wk, Wv, Wp, bp, T, DT_NAME)
    res = run_bass_kernel_spmd(nc, in_maps, list(range(N_CORES)))
    return np.ascontiguousarray(
        gather_out(res.results, T).astype(np.float32))
